# revision 1
# baseline (speedup 1.0000x reference)
"""Distributed Trainium2 kernel for nn_ContrastiveLoss (survival contrastive loss).

Strategy (8 NeuronCores, data-parallel over rows):
  host: quantile-bin rows into 4 risk groups, stable-sort rows by group,
        transpose embeddings to [D, N]; each core gets a rotated copy so
        its own 1024 rows sit at columns 0..1023 (static APs, SPMD-safe).
  device (per core): normalize columns (ssq via ones-matmul broadcast,
        sqrt, reciprocal, scale) -> z^T in f32r; for each 128-row block
        compute sim row-block via f32r matmuls (K=512 in 4 chunks),
        fused exp(10*sim-10)+row-sum on the scalar engine per 512-col
        tile; group sums = masked sums over whole tiles (groups are
        2048-aligned after sorting); subtract the exact diagonal term,
        log-ratio -> per-row loss.
  host: sum per-row losses / N.
"""
import sys

sys.path.insert(0, "/opt/trn_rl_repo")
import numpy as np

N, D, G, NCORES = 8192, 512, 4, 8
TEMP = 0.1
RPC = N // NCORES      # 1024 rows per core
RB = RPC // 128        # 8 row blocks per core
CT = 512               # column tile
NT = N // CT           # 16 column tiles
KC = D // 128          # 4 contraction chunks

_built = None


def _build():
    from concourse import bacc, tile, mybir

    nc = bacc.Bacc(None, target_bir_lowering=False)
    f32 = mybir.dt.float32
    f32r = mybir.dt.float32r
    AF = mybir.ActivationFunctionType
    AX = mybir.AxisListType

    et = nc.dram_tensor("et", [D, N], f32, kind="ExternalInput")
    sel = nc.dram_tensor("sel", [128, RB * NT], f32, kind="ExternalInput")
    dmask = nc.dram_tensor("dmask", [128, 4 * CT], f32, kind="ExternalInput")
    ones = nc.dram_tensor("ones", [128, 128], f32, kind="ExternalInput")
    pr = nc.dram_tensor("pr", [128, RB], f32, kind="ExternalOutput")

    with tile.TileContext(nc) as tc:
        with tc.tile_pool(name="zt", bufs=1) as ztp, \
             tc.tile_pool(name="cst", bufs=1) as cst, \
             tc.tile_pool(name="io", bufs=2) as io, \
             tc.tile_pool(name="eb", bufs=6) as ebp, \
             tc.tile_pool(name="sm", bufs=3) as smp, \
             tc.tile_pool(name="pp", bufs=2, space="PSUM") as ppp, \
             tc.tile_pool(name="pm", bufs=6, space="PSUM") as pmp:

            # preload the exp+ln activation table once; all ACT funcs used
            # below (Exp, Ln) live in set 6 = natural_log_exp_and_others,
            # so the act-table fixpoint pass inserts no further reloads
            nc.scalar.add_instruction(
                mybir.InstLoadActFuncSet(
                    name=nc.get_next_instruction_name(),
                    act_func_set_id=6, ins=[], outs=[]))

            onest = cst.tile([128, 128], f32r)
            nc.sync.dma_start(onest[:], ones[:].bitcast(f32r))
            dmt = cst.tile([128, 4 * CT], f32)
            nc.sync.dma_start(dmt[:], dmask[:])
            selt = cst.tile([128, RB * NT], f32)
            nc.sync.dma_start(selt[:], sel[:])
            bias10 = cst.tile([128, 1], f32)
            nc.vector.memset(bias10[:], -10.0)
            prt = cst.tile([128, RB], f32)

            zts = [ztp.tile([128, N], f32r, tag=f"zt{k}", name=f"zt{k}")
                   for k in range(KC)]

            # ---- prep: column norms + scale -> z^T (f32r) ----
            for c in range(NT):
                cs = slice(c * CT, (c + 1) * CT)
                chunks = []
                for k in range(KC):
                    ch = io.tile([128, CT], f32, tag=f"ch{k}")
                    nc.sync.dma_start(ch[:], et[k * 128:(k + 1) * 128, cs])
                    chunks.append(ch)
                ps = ppp.tile([128, CT], f32)
                for k in range(KC):
                    sq = io.tile([128, CT], f32r, tag=f"sq{k}")
                    # split squares across DVE and ACT: prep is DVE-bound
                    # while ACT idles (Square is in table set 6 -> no reload)
                    if k < 2:
                        nc.vector.tensor_mul(sq[:], chunks[k][:], chunks[k][:])
                    else:
                        nc.scalar.activation(sq[:], chunks[k][:], AF.Square)
                    nc.tensor.matmul(ps[:], onest[:], sq[:],
                                     start=(k == 0), stop=(k == KC - 1))
                # rsqrt(ssq) = exp(-0.5*ln(ssq)): keeps every ACT op inside
                # the natural_log_exp_and_others table set (no table reloads)
                st = io.tile([128, CT], f32, tag="st")
                nc.scalar.activation(st[:], ps[:], AF.Ln)
                rbt = io.tile([128, CT], f32, tag="rbt")
                nc.scalar.activation(rbt[:], st[:], AF.Exp, scale=-0.5)
                for k in range(KC):
                    nc.vector.tensor_mul(zts[k][:, cs], chunks[k][:], rbt[:])

            # ---- main: per row-block masked logsumexp sums ----
            dens = cst.tile([128, RB], f32)
            poss = cst.tile([128, RB], f32)
            for r in range(RB):
                rs = slice(r * 128, (r + 1) * 128)
                ssc = smp.tile([128, NT], f32, tag="ssc")
                dval = smp.tile([128, 1], f32, tag="dval")
                for t in range(NT):
                    ts = slice(t * CT, (t + 1) * CT)
                    pm = pmp.tile([128, CT], f32)
                    for k in range(KC):
                        nc.tensor.matmul(pm[:], zts[k][:, rs], zts[k][:, ts],
                                         start=(k == 0), stop=(k == KC - 1))
                    eb = ebp.tile([128, CT], f32, tag="eb")
                    nc.scalar.activation(eb[:], pm[:], AF.Exp,
                                         bias=bias10[:], scale=1.0 / TEMP,
                                         accum_out=ssc[:, t:t + 1])
                    if t == r // 4:
                        o = r % 4
                        dt_ = ebp.tile([128, CT], f32, tag="dtmp")
                        nc.vector.tensor_mul(
                            dt_[:], eb[:], dmt[:, o * CT:(o + 1) * CT])
                        nc.vector.reduce_sum(dval[:], dt_[:], axis=AX.X)
                sall = smp.tile([128, 1], f32, tag="sall")
                nc.vector.reduce_sum(sall[:], ssc[:], axis=AX.X)
                spm = smp.tile([128, NT], f32, tag="spm")
                nc.vector.tensor_mul(spm[:], ssc[:],
                                     selt[:, r * NT:(r + 1) * NT])
                spos = smp.tile([128, 1], f32, tag="spos")
                nc.vector.reduce_sum(spos[:], spm[:], axis=AX.X)
                nc.vector.tensor_sub(dens[:, r:r + 1], sall[:], dval[:])
                nc.vector.tensor_sub(poss[:, r:r + 1], spos[:], dval[:])

            # batched tail: 2 Ln + 1 sub for all row blocks
            ldens = cst.tile([128, RB], f32)
            nc.scalar.activation(ldens[:], dens[:], AF.Ln)
            lposs = cst.tile([128, RB], f32)
            nc.scalar.activation(lposs[:], poss[:], AF.Ln)
            nc.vector.tensor_sub(prt[:], ldens[:], lposs[:])

            nc.sync.dma_start(pr[:], prt[:])

    nc.finalize()
    return nc


def _get_built():
    global _built
    if _built is None:
        _built = _build()
    return _built


def _host_prep(embeddings, survival_times):
    E = np.ascontiguousarray(np.asarray(embeddings, dtype=np.float32))
    t = np.asarray(survival_times, dtype=np.float32)
    q = np.quantile(t.astype(np.float64), [0.25, 0.5, 0.75])
    rg = (t[:, None].astype(np.float64) >= q[None, :]).sum(axis=1)
    counts = np.bincount(rg, minlength=G)
    # layout assumptions: every group is a whole number of 512-col tiles
    # and every 128-row block is within one group (true for quantile bins
    # of N=8192 distinct values: 2048 per group)
    assert (counts % CT == 0).all() and (counts >= 2).all(), counts
    perm = np.argsort(rg, kind="stable")
    ET = np.ascontiguousarray(E[perm].T)  # [D, N]
    bounds = np.concatenate([[0], np.cumsum(counts)])
    gcol_global = np.searchsorted(bounds, np.arange(NT) * CT, side="right") - 1
    grow_global = np.searchsorted(bounds, np.arange(N // 128) * 128,
                                  side="right") - 1

    dmask = np.zeros((128, 4 * CT), dtype=np.float32)
    for o in range(4):
        for p in range(128):
            dmask[p, o * CT + o * 128 + p] = 1.0
    ones = np.ones((128, 128), dtype=np.float32)

    in_maps = []
    for k in range(NCORES):
        et_k = np.ascontiguousarray(np.roll(ET, -k * RPC, axis=1))
        sel_k = np.zeros((128, RB * NT), dtype=np.float32)
        for r in range(RB):
            g_row = grow_global[(k * RPC + r * 128) // 128]
            for tt in range(NT):
                gc = gcol_global[((tt * CT + k * RPC) % N) // CT]
                if gc == g_row:
                    sel_k[:, r * NT + tt] = 1.0
        in_maps.append({"et": et_k, "sel": sel_k, "dmask": dmask,
                        "ones": ones})
    return in_maps


def kernel(embeddings, survival_times, censor):
    from concourse.bass_utils import run_bass_kernel_spmd

    nc = _get_built()
    in_maps = _host_prep(embeddings, survival_times)
    res = run_bass_kernel_spmd(nc, in_maps, list(range(NCORES)))
    total = 0.0
    for i in range(NCORES):
        total += res.results[i]["pr"].astype(np.float64).sum()
    return np.float32(total / N)



# revision 7
# speedup vs baseline: 3.2089x; 3.2089x over previous
"""Distributed Trainium2 kernel for nn_ContrastiveLoss (survival contrastive loss).

Strategy (8 NeuronCores, symmetric fp8):
  host: quantile-bin rows into 4 risk groups (2048 each), stable-sort by
        group, L2-normalize, scale by 16 and cast to fp8e4 (e4m3); ship a
        rolled copy to each core so its supertile-rows sit at fixed virtual
        positions (SPMD-static program).
  device (core c): sim is symmetric, so only supertile pairs (I, I+d) for
        virtual I in {0,8}, d = 0..8 / 0..7 are computed — over 8 rolled
        copies this covers all 136 unordered 512x512 supertile pairs once.
        fp8 DoubleRow matmuls (K=256/matmul) -> psum; ACT exp (scale 10/256)
        in 3-tile batches with f32 accum row-sums; fp8 exp tiles feed
        DoubleRow ones-matmul column-sums (the mirror contribution) and a
        DVE reduce of tiles d=1..3 (group-boundary corrections).
  host: assemble per-row pos/den sums from row-accums, boundary reduces and
        colsums; subtract the exact diagonal exp(10*||z8||^2/256) computed
        from the shipped fp8 values; loss = mean(log den - log pos).
"""
import sys

sys.path.insert(0, "/opt/trn_rl_repo")
import numpy as np
import ml_dtypes

N, D, G, NCORES = 8192, 512, 4, 8
CT = 512               # supertile width
NT = N // CT           # 16 supertiles
SCALE = 16.0           # fp8 pre-scale
ESC = 10.0 / (SCALE * SCALE)   # exp scale applied to psum
F8NP = ml_dtypes.float8_e4m3

_built = None


def _build():
    from concourse import bacc, tile, mybir

    nc = bacc.Bacc(None, target_bir_lowering=False)
    f32 = mybir.dt.float32
    f8 = mybir.dt.float8e4
    AF = mybir.ActivationFunctionType
    AX = mybir.AxisListType
    PM = mybir.MatmulPerfMode.DoubleRow

    et = nc.dram_tensor("et", [128, 4, N], f8, kind="ExternalInput")
    ones2 = nc.dram_tensor("ones2", [128, 2, 16], f8, kind="ExternalInput")
    racc = nc.dram_tensor("racc", [128, 24], f32, kind="ExternalOutput")
    rred = nc.dram_tensor("rred", [128, 24], f32, kind="ExternalOutput")
    csum = nc.dram_tensor("csum", [15, 512], f32, kind="ExternalOutput")

    with tile.TileContext(nc) as tc:
        with tc.tile_pool(name="z", bufs=1) as zp, \
             tc.tile_pool(name="cst", bufs=1) as cst, \
             tc.tile_pool(name="eb", bufs=2) as ebp, \
             tc.tile_pool(name="pm", bufs=2, space="PSUM") as pmp, \
             tc.tile_pool(name="pc", bufs=2, space="PSUM") as pcp:

            nc.scalar.add_instruction(
                mybir.InstLoadActFuncSet(
                    name=nc.get_next_instruction_name(),
                    act_func_set_id=6, ins=[], outs=[]))

            z8 = zp.tile([128, 4, N], f8)
            # front columns first: supertile-row I=0 only needs cols < 9*512
            nc.sync.dma_start(z8[:, :, :9 * CT], et[:, :, :9 * CT])
            nc.sync.dma_start(z8[:, :, 9 * CT:], et[:, :, 9 * CT:])
            o2 = cst.tile([128, 2, 16], f8)
            nc.sync.dma_start(o2[:], ones2[:])
            racc_t = cst.tile([128, 24], f32)
            rred_t = cst.tile([128, 24], f32)
            cstage = cst.tile([1, 15 * CT], f32)

            s = 0
            for Ii, I in enumerate((0, 8)):
                maxd = 9 if I == 0 else 8
                batches = [(0, 1, 2), (3, 4, 5),
                           (6, 7, 8) if I == 0 else (6, 7)]
                # exp tiles: [rb, d, col] fp8
                expt = ebp.tile([128, 4, 9, CT], f8, tag="expt")
                for rb in range(4):
                    rs = slice(I * CT + rb * 128, I * CT + (rb + 1) * 128)
                    for b, ds in enumerate(batches):
                        pm = pmp.tile([128, 3 * CT], f32, tag="pm")
                        for kk in range(2):
                            for di, d in enumerate(ds):
                                cs = slice((I + d) * CT, (I + d + 1) * CT)
                                nc.tensor.matmul(
                                    pm[:, di * CT:(di + 1) * CT],
                                    z8[:, 2 * kk:2 * kk + 2, rs],
                                    z8[:, 2 * kk:2 * kk + 2, cs],
                                    start=(kk == 0), stop=(kk == 1),
                                    perf_mode=PM)
                        acol = Ii * 12 + rb * 3 + b
                        nc.scalar.activation(
                            expt[:, rb, ds[0]:ds[0] + len(ds), :],
                            pm[:, :len(ds) * CT], AF.Exp, scale=ESC,
                            accum_out=racc_t[:, acol:acol + 1])
                    # rowsums of fp8 tiles d=1,2,3 (group-boundary info)
                    rcol = Ii * 12 + rb * 3
                    nc.vector.tensor_reduce(
                        rred_t[:, rcol:rcol + 3], expt[:, rb, 1:4, :],
                        axis=AX.X, op=mybir.AluOpType.add)
                for d in range(1, maxd):
                    pc = pcp.tile([1, CT], f32, tag="pc")
                    for h in range(2):
                        nc.tensor.matmul(
                            pc[:], o2[:, :, 0:1],
                            expt[:, 2 * h:2 * h + 2, d, :],
                            start=(h == 0), stop=(h == 1), perf_mode=PM)
                    nc.vector.tensor_copy(cstage[:, s * CT:(s + 1) * CT], pc[:])
                    s += 1
            assert s == 15
            nc.sync.dma_start(csum[:], cstage[:])
            nc.sync.dma_start(racc[:], racc_t[:])
            nc.sync.dma_start(rred[:], rred_t[:])

    nc.finalize()
    return nc


def _get_built():
    global _built
    if _built is None:
        _built = _build()
    return _built


def _host_prep(embeddings, survival_times):
    E = np.ascontiguousarray(np.asarray(embeddings, dtype=np.float32))
    t = np.asarray(survival_times, dtype=np.float32)
    q = np.quantile(t.astype(np.float64), [0.25, 0.5, 0.75])
    rg = (t[:, None].astype(np.float64) >= q[None, :]).sum(axis=1)
    counts = np.bincount(rg, minlength=G)
    assert (counts == N // G).all(), counts
    perm = np.argsort(rg, kind="stable")
    Es = E[perm]
    nrm = np.sqrt((Es.astype(np.float64) ** 2).sum(axis=1, keepdims=True))
    z = Es / np.maximum(nrm, 1e-12)
    z16 = (z * SCALE).astype(F8NP)          # [N, D] fp8
    zT = np.ascontiguousarray(z16.T)        # [D, N]
    ones2 = np.zeros((128, 2, 16), dtype=F8NP)
    ones2[:, :, 0] = 1.0
    in_maps = []
    for c in range(NCORES):
        roll = np.roll(zT, -c * CT, axis=1)               # [D, N]
        et = np.ascontiguousarray(
            roll.reshape(4, 128, N).transpose(1, 0, 2))    # [128, 4, N]
        in_maps.append({"et": et, "ones2": ones2})
    return in_maps, z16


def _host_combine(results, z16):
    tot = np.zeros(N, np.float64)
    pos = np.zeros(N, np.float64)
    for c in range(NCORES):
        racc = results[c]["racc"].astype(np.float64)
        rred = results[c]["rred"].astype(np.float64)
        csum = results[c]["csum"].astype(np.float64)
        s = 0
        for Ii, I in enumerate((0, 8)):
            aI = (I + c) % NT
            maxd = 9 if I == 0 else 8
            gI = aI // 4
            kp = 4 - (aI % 4)
            for rb in range(4):
                rows = slice(aI * CT + rb * 128, aI * CT + (rb + 1) * 128)
                A = racc[:, Ii * 12 + rb * 3: Ii * 12 + rb * 3 + 3]
                R = rred[:, Ii * 12 + rb * 3: Ii * 12 + rb * 3 + 3]
                tot[rows] += A.sum(axis=1)
                if kp == 1:
                    p = A[:, 0] - R[:, 0] - R[:, 1]
                elif kp == 2:
                    p = A[:, 0] - R[:, 1]
                elif kp == 3:
                    p = A[:, 0]
                else:
                    p = A[:, 0] + R[:, 2]
                pos[rows] += p
            for d in range(1, maxd):
                aJ = (I + d + c) % NT
                rows = slice(aJ * CT, (aJ + 1) * CT)
                tot[rows] += csum[s]
                if aJ // 4 == gI:
                    pos[rows] += csum[s]
                s += 1
    dlog = ESC * (z16.astype(np.float64) ** 2).sum(axis=1)
    dexp = np.exp(dlog)
    tot -= dexp
    pos -= dexp
    return np.float32(np.mean(np.log(tot) - np.log(pos)))


def kernel(embeddings, survival_times, censor):
    from concourse.bass_utils import run_bass_kernel_spmd

    nc = _get_built()
    in_maps, z16 = _host_prep(embeddings, survival_times)
    res = run_bass_kernel_spmd(nc, in_maps, list(range(NCORES)))
    return _host_combine(res.results, z16)


# revision 9
# speedup vs baseline: 3.5936x; 1.1199x over previous
"""Distributed Trainium2 kernel for nn_ContrastiveLoss (survival contrastive loss).

Strategy (8 NeuronCores, symmetric fp8):
  host: quantile-bin rows into 4 risk groups (2048 each), stable-sort by
        group, L2-normalize, scale by 16 and cast to fp8e4 (e4m3); ship a
        rolled copy to each core so its supertile-rows sit at fixed virtual
        positions (SPMD-static program).
  device (core c): sim is symmetric, so only supertile pairs (I, I+d) for
        virtual I in {0,8}, d = 0..8 / 0..7 are computed — over 8 rolled
        copies this covers all 136 unordered 512x512 supertile pairs once.
        fp8 DoubleRow matmuls (K=256/matmul) -> psum; ACT exp (scale 10/256)
        in 3-tile batches with f32 accum row-sums; fp8 exp tiles feed
        DoubleRow ones-matmul column-sums (the mirror contribution) and a
        DVE reduce of tiles d=1..3 (group-boundary corrections).
  host: assemble per-row pos/den sums from row-accums, boundary reduces and
        colsums; subtract the exact diagonal exp(10*||z8||^2/256) computed
        from the shipped fp8 values; loss = mean(log den - log pos).
"""
import sys

sys.path.insert(0, "/opt/trn_rl_repo")
import numpy as np
import ml_dtypes

N, D, G, NCORES = 8192, 512, 4, 8
CT = 512               # supertile width
NT = N // CT           # 16 supertiles
SCALE = 16.0           # fp8 pre-scale
ESC = 10.0 / (SCALE * SCALE)   # exp scale applied to psum
F8NP = ml_dtypes.float8_e4m3

_built = None


def _build():
    from concourse import bacc, tile, mybir

    nc = bacc.Bacc(None, target_bir_lowering=False)
    f32 = mybir.dt.float32
    f8 = mybir.dt.float8e4
    AF = mybir.ActivationFunctionType
    AX = mybir.AxisListType
    PM = mybir.MatmulPerfMode.DoubleRow

    et = nc.dram_tensor("et", [128, 4, N], f8, kind="ExternalInput")
    ones2 = nc.dram_tensor("ones2", [128, 2, 16], f8, kind="ExternalInput")
    racc = nc.dram_tensor("racc", [128, 24], f32, kind="ExternalOutput")
    rred = nc.dram_tensor("rred", [128, 24], f32, kind="ExternalOutput")
    csum = nc.dram_tensor("csum", [15, 512], f32, kind="ExternalOutput")

    with tile.TileContext(nc) as tc:
        with tc.tile_pool(name="z", bufs=1) as zp, \
             tc.tile_pool(name="cst", bufs=1) as cst, \
             tc.tile_pool(name="eb", bufs=2) as ebp, \
             tc.tile_pool(name="pm", bufs=2, space="PSUM") as pmp, \
             tc.tile_pool(name="pc", bufs=2, space="PSUM") as pcp:

            nc.scalar.add_instruction(
                mybir.InstLoadActFuncSet(
                    name=nc.get_next_instruction_name(),
                    act_func_set_id=6, ins=[], outs=[]))

            o2 = cst.tile([128, 2, 16], f8)
            nc.sync.dma_start(o2[:], ones2[:])
            z8 = zp.tile([128, 4, N], f8)
            # progressive column chunks so compute starts after ~2.4us
            bounds = [0, 3 * CT, 6 * CT, 9 * CT, 12 * CT, 16 * CT]
            for lo, hi in zip(bounds, bounds[1:]):
                nc.sync.dma_start(z8[:, :, lo:hi], et[:, :, lo:hi])
            racc_t = cst.tile([128, 24], f32)
            rred_t = cst.tile([128, 24], f32)
            cstage = cst.tile([1, 15 * CT], f32)

            s = 0
            for Ii, I in enumerate((0, 8)):
                maxd = 9 if I == 0 else 8
                batches = [(0, 1, 2), (3, 4, 5),
                           (6, 7, 8) if I == 0 else (6, 7)]
                # exp tiles: [rb, d, col] fp8
                expt = ebp.tile([128, 4, 9, CT], f8, tag="expt")
                # batch-major: only the first pass over rb waits on new
                # input columns, later batches reuse columns already loaded
                for b, ds in enumerate(batches):
                    for rb in range(4):
                        rs = slice(I * CT + rb * 128, I * CT + (rb + 1) * 128)
                        pm = pmp.tile([128, 3 * CT], f32, tag="pm")
                        for kk in range(2):
                            for di, d in enumerate(ds):
                                cs = slice((I + d) * CT, (I + d + 1) * CT)
                                nc.tensor.matmul(
                                    pm[:, di * CT:(di + 1) * CT],
                                    z8[:, 2 * kk:2 * kk + 2, rs],
                                    z8[:, 2 * kk:2 * kk + 2, cs],
                                    start=(kk == 0), stop=(kk == 1),
                                    perf_mode=PM)
                        acol = Ii * 12 + rb * 3 + b
                        nc.scalar.activation(
                            expt[:, rb, ds[0]:ds[0] + len(ds), :],
                            pm[:, :len(ds) * CT], AF.Exp, scale=ESC,
                            accum_out=racc_t[:, acol:acol + 1])
                    if b == 1:
                        # d=1..3 fp8 rowsums (group-boundary info)
                        for rb in range(4):
                            rcol = Ii * 12 + rb * 3
                            nc.vector.tensor_reduce(
                                rred_t[:, rcol:rcol + 3], expt[:, rb, 1:4, :],
                                axis=AX.X, op=mybir.AluOpType.add)
                    # colsums for completed off-diag tiles of this batch
                    for d in ds:
                        if d == 0:
                            continue
                        pc = pcp.tile([1, CT], f32, tag="pc")
                        for h in range(2):
                            nc.tensor.matmul(
                                pc[:], o2[:, :, 0:1],
                                expt[:, 2 * h:2 * h + 2, d, :],
                                start=(h == 0), stop=(h == 1), perf_mode=PM)
                        nc.vector.tensor_copy(
                            cstage[:, s * CT:(s + 1) * CT], pc[:])
                        s += 1
            assert s == 15
            nc.sync.dma_start(csum[:], cstage[:])
            nc.sync.dma_start(racc[:], racc_t[:])
            nc.sync.dma_start(rred[:], rred_t[:])

    nc.finalize()
    return nc


def _get_built():
    global _built
    if _built is None:
        _built = _build()
    return _built


def _host_prep(embeddings, survival_times):
    E = np.ascontiguousarray(np.asarray(embeddings, dtype=np.float32))
    t = np.asarray(survival_times, dtype=np.float32)
    q = np.quantile(t.astype(np.float64), [0.25, 0.5, 0.75])
    rg = (t[:, None].astype(np.float64) >= q[None, :]).sum(axis=1)
    counts = np.bincount(rg, minlength=G)
    assert (counts == N // G).all(), counts
    perm = np.argsort(rg, kind="stable")
    Es = E[perm]
    nrm = np.sqrt((Es.astype(np.float64) ** 2).sum(axis=1, keepdims=True))
    z = Es / np.maximum(nrm, 1e-12)
    z16 = (z * SCALE).astype(F8NP)          # [N, D] fp8
    zT = np.ascontiguousarray(z16.T)        # [D, N]
    ones2 = np.zeros((128, 2, 16), dtype=F8NP)
    ones2[:, :, 0] = 1.0
    in_maps = []
    for c in range(NCORES):
        roll = np.roll(zT, -c * CT, axis=1)               # [D, N]
        et = np.ascontiguousarray(
            roll.reshape(4, 128, N).transpose(1, 0, 2))    # [128, 4, N]
        in_maps.append({"et": et, "ones2": ones2})
    return in_maps, z16


def _host_combine(results, z16):
    tot = np.zeros(N, np.float64)
    pos = np.zeros(N, np.float64)
    for c in range(NCORES):
        racc = results[c]["racc"].astype(np.float64)
        rred = results[c]["rred"].astype(np.float64)
        csum = results[c]["csum"].astype(np.float64)
        s = 0
        for Ii, I in enumerate((0, 8)):
            aI = (I + c) % NT
            maxd = 9 if I == 0 else 8
            gI = aI // 4
            kp = 4 - (aI % 4)
            for rb in range(4):
                rows = slice(aI * CT + rb * 128, aI * CT + (rb + 1) * 128)
                A = racc[:, Ii * 12 + rb * 3: Ii * 12 + rb * 3 + 3]
                R = rred[:, Ii * 12 + rb * 3: Ii * 12 + rb * 3 + 3]
                tot[rows] += A.sum(axis=1)
                if kp == 1:
                    p = A[:, 0] - R[:, 0] - R[:, 1]
                elif kp == 2:
                    p = A[:, 0] - R[:, 1]
                elif kp == 3:
                    p = A[:, 0]
                else:
                    p = A[:, 0] + R[:, 2]
                pos[rows] += p
            for d in range(1, maxd):
                aJ = (I + d + c) % NT
                rows = slice(aJ * CT, (aJ + 1) * CT)
                tot[rows] += csum[s]
                if aJ // 4 == gI:
                    pos[rows] += csum[s]
                s += 1
    dlog = ESC * (z16.astype(np.float64) ** 2).sum(axis=1)
    dexp = np.exp(dlog)
    tot -= dexp
    pos -= dexp
    return np.float32(np.mean(np.log(tot) - np.log(pos)))


def kernel(embeddings, survival_times, censor):
    from concourse.bass_utils import run_bass_kernel_spmd

    nc = _get_built()
    in_maps, z16 = _host_prep(embeddings, survival_times)
    res = run_bass_kernel_spmd(nc, in_maps, list(range(NCORES)))
    return _host_combine(res.results, z16)


# revision 10
# speedup vs baseline: 3.6725x; 1.0220x over previous
"""Distributed Trainium2 kernel for nn_ContrastiveLoss (survival contrastive loss).

Strategy (8 NeuronCores, symmetric fp8):
  host: quantile-bin rows into 4 risk groups (2048 each), stable-sort by
        group, L2-normalize, scale by 16 and cast to fp8e4 (e4m3); ship a
        rolled copy to each core so its supertile-rows sit at fixed virtual
        positions (SPMD-static program).
  device (core c): sim is symmetric, so only supertile pairs (I, I+d) for
        virtual I in {0,8}, d = 0..8 / 0..7 are computed — over 8 rolled
        copies this covers all 136 unordered 512x512 supertile pairs once.
        fp8 DoubleRow matmuls (K=256/matmul) -> psum; ACT exp (scale 10/256)
        in 3-tile batches with f32 accum row-sums; fp8 exp tiles feed
        DoubleRow ones-matmul column-sums (the mirror contribution) and a
        DVE reduce of tiles d=1..3 (group-boundary corrections).
  host: assemble per-row pos/den sums from row-accums, boundary reduces and
        colsums; subtract the exact diagonal exp(10*||z8||^2/256) computed
        from the shipped fp8 values; loss = mean(log den - log pos).
"""
import sys

sys.path.insert(0, "/opt/trn_rl_repo")
import numpy as np
import ml_dtypes

N, D, G, NCORES = 8192, 512, 4, 8
CT = 512               # supertile width
NT = N // CT           # 16 supertiles
SCALE = 16.0           # fp8 pre-scale
ESC = 10.0 / (SCALE * SCALE)   # exp scale applied to psum
F8NP = ml_dtypes.float8_e4m3

_built = None


def _build():
    from concourse import bacc, tile, mybir

    nc = bacc.Bacc(None, target_bir_lowering=False)
    f32 = mybir.dt.float32
    f8 = mybir.dt.float8e4
    AF = mybir.ActivationFunctionType
    AX = mybir.AxisListType
    PM = mybir.MatmulPerfMode.DoubleRow

    et = nc.dram_tensor("et", [128, 4, N], f8, kind="ExternalInput")
    ones2 = nc.dram_tensor("ones2", [128, 2, 16], f8, kind="ExternalInput")
    rsums = nc.dram_tensor("rsums", [128, 48], f32, kind="ExternalOutput")
    csum = nc.dram_tensor("csum", [15, 512], f32, kind="ExternalOutput")

    with tile.TileContext(nc) as tc:
        with tc.tile_pool(name="z", bufs=1) as zp, \
             tc.tile_pool(name="cst", bufs=1) as cst, \
             tc.tile_pool(name="eb", bufs=2) as ebp, \
             tc.tile_pool(name="pm", bufs=2, space="PSUM") as pmp, \
             tc.tile_pool(name="pc", bufs=2, space="PSUM") as pcp:

            nc.scalar.add_instruction(
                mybir.InstLoadActFuncSet(
                    name=nc.get_next_instruction_name(),
                    act_func_set_id=6, ins=[], outs=[]))

            o2 = cst.tile([128, 2, 16], f8)
            nc.sync.dma_start(o2[:], ones2[:])
            z8 = zp.tile([128, 4, N], f8)
            # progressive column chunks so compute starts after ~2.4us
            bounds = [0, CT, 2 * CT, 3 * CT, 6 * CT, 9 * CT, 12 * CT, 16 * CT]
            for lo, hi in zip(bounds, bounds[1:]):
                nc.sync.dma_start(z8[:, :, lo:hi], et[:, :, lo:hi])
            rsums_t = cst.tile([128, 48], f32)
            cstage = cst.tile([1, 15 * CT], f32)

            s = 0
            for Ii, I in enumerate((0, 8)):
                maxd = 9 if I == 0 else 8
                batches = [(0, 1, 2), (3, 4, 5),
                           (6, 7, 8) if I == 0 else (6, 7)]
                # exp tiles: [rb, d, col] fp8
                expt = ebp.tile([128, 4, 9, CT], f8, tag="expt")
                # batch-major: only the first pass over rb waits on new
                # input columns, later batches reuse columns already loaded
                for b, ds in enumerate(batches):
                    for rb in range(4):
                        rs = slice(I * CT + rb * 128, I * CT + (rb + 1) * 128)
                        pm = pmp.tile([128, 3 * CT], f32, tag="pm")
                        for di, d in enumerate(ds):
                            cs = slice((I + d) * CT, (I + d + 1) * CT)
                            for kk in range(2):
                                nc.tensor.matmul(
                                    pm[:, di * CT:(di + 1) * CT],
                                    z8[:, 2 * kk:2 * kk + 2, rs],
                                    z8[:, 2 * kk:2 * kk + 2, cs],
                                    start=(kk == 0), stop=(kk == 1),
                                    perf_mode=PM)
                        acol = Ii * 12 + rb * 3 + b
                        nc.scalar.activation(
                            expt[:, rb, ds[0]:ds[0] + len(ds), :],
                            pm[:, :len(ds) * CT], AF.Exp, scale=ESC,
                            accum_out=rsums_t[:, acol:acol + 1])
                    if b == 1:
                        # d=1..3 fp8 rowsums (group-boundary info)
                        for rb in range(4):
                            rcol = Ii * 12 + rb * 3
                            nc.vector.tensor_reduce(
                                rsums_t[:, 24 + rcol:24 + rcol + 3],
                                expt[:, rb, 1:4, :],
                                axis=AX.X, op=mybir.AluOpType.add)
                    # colsums for completed off-diag tiles of this batch
                    for d in ds:
                        if d == 0:
                            continue
                        pc = pcp.tile([1, CT], f32, tag="pc")
                        for h in range(2):
                            nc.tensor.matmul(
                                pc[:], o2[:, :, 0:1],
                                expt[:, 2 * h:2 * h + 2, d, :],
                                start=(h == 0), stop=(h == 1), perf_mode=PM)
                        nc.vector.tensor_copy(
                            cstage[:, s * CT:(s + 1) * CT], pc[:])
                        s += 1
                        if s == 13:
                            nc.sync.dma_start(csum[:13, :],
                                              cstage[:, :13 * CT])
            assert s == 15
            nc.sync.dma_start(csum[13:, :], cstage[:, 13 * CT:])
            nc.sync.dma_start(rsums[:], rsums_t[:])

    nc.finalize()
    return nc


def _get_built():
    global _built
    if _built is None:
        _built = _build()
    return _built


def _host_prep(embeddings, survival_times):
    E = np.ascontiguousarray(np.asarray(embeddings, dtype=np.float32))
    t = np.asarray(survival_times, dtype=np.float32)
    q = np.quantile(t.astype(np.float64), [0.25, 0.5, 0.75])
    rg = (t[:, None].astype(np.float64) >= q[None, :]).sum(axis=1)
    counts = np.bincount(rg, minlength=G)
    assert (counts == N // G).all(), counts
    perm = np.argsort(rg, kind="stable")
    Es = E[perm]
    nrm = np.sqrt((Es.astype(np.float64) ** 2).sum(axis=1, keepdims=True))
    z = Es / np.maximum(nrm, 1e-12)
    z16 = (z * SCALE).astype(F8NP)          # [N, D] fp8
    zT = np.ascontiguousarray(z16.T)        # [D, N]
    ones2 = np.zeros((128, 2, 16), dtype=F8NP)
    ones2[:, :, 0] = 1.0
    in_maps = []
    for c in range(NCORES):
        roll = np.roll(zT, -c * CT, axis=1)               # [D, N]
        et = np.ascontiguousarray(
            roll.reshape(4, 128, N).transpose(1, 0, 2))    # [128, 4, N]
        in_maps.append({"et": et, "ones2": ones2})
    return in_maps, z16


def _host_combine(results, z16):
    tot = np.zeros(N, np.float64)
    pos = np.zeros(N, np.float64)
    for c in range(NCORES):
        rs_ = results[c]["rsums"].astype(np.float64)
        racc, rred = rs_[:, :24], rs_[:, 24:]
        csum = results[c]["csum"].astype(np.float64)
        s = 0
        for Ii, I in enumerate((0, 8)):
            aI = (I + c) % NT
            maxd = 9 if I == 0 else 8
            gI = aI // 4
            kp = 4 - (aI % 4)
            for rb in range(4):
                rows = slice(aI * CT + rb * 128, aI * CT + (rb + 1) * 128)
                A = racc[:, Ii * 12 + rb * 3: Ii * 12 + rb * 3 + 3]
                R = rred[:, Ii * 12 + rb * 3: Ii * 12 + rb * 3 + 3]
                tot[rows] += A.sum(axis=1)
                if kp == 1:
                    p = A[:, 0] - R[:, 0] - R[:, 1]
                elif kp == 2:
                    p = A[:, 0] - R[:, 1]
                elif kp == 3:
                    p = A[:, 0]
                else:
                    p = A[:, 0] + R[:, 2]
                pos[rows] += p
            for d in range(1, maxd):
                aJ = (I + d + c) % NT
                rows = slice(aJ * CT, (aJ + 1) * CT)
                tot[rows] += csum[s]
                if aJ // 4 == gI:
                    pos[rows] += csum[s]
                s += 1
    dlog = ESC * (z16.astype(np.float64) ** 2).sum(axis=1)
    dexp = np.exp(dlog)
    tot -= dexp
    pos -= dexp
    return np.float32(np.mean(np.log(tot) - np.log(pos)))


def kernel(embeddings, survival_times, censor):
    from concourse.bass_utils import run_bass_kernel_spmd

    nc = _get_built()
    in_maps, z16 = _host_prep(embeddings, survival_times)
    res = run_bass_kernel_spmd(nc, in_maps, list(range(NCORES)))
    return _host_combine(res.results, z16)


# revision 11
# speedup vs baseline: 3.6820x; 1.0026x over previous
"""Distributed Trainium2 kernel for nn_ContrastiveLoss (survival contrastive loss).

Strategy (8 NeuronCores, symmetric fp8):
  host: quantile-bin rows into 4 risk groups (2048 each), stable-sort by
        group, L2-normalize, scale by 16 and cast to fp8e4 (e4m3); ship a
        rolled copy to each core so its supertile-rows sit at fixed virtual
        positions (SPMD-static program).
  device (core c): sim is symmetric, so only supertile pairs (I, I+d) for
        virtual I in {0,8}, d = 0..8 / 0..7 are computed — over 8 rolled
        copies this covers all 136 unordered 512x512 supertile pairs once.
        fp8 DoubleRow matmuls (K=256/matmul) -> psum; ACT exp (scale 10/256)
        in 3-tile batches with f32 accum row-sums; fp8 exp tiles feed
        DoubleRow ones-matmul column-sums (the mirror contribution) and a
        DVE reduce of tiles d=1..3 (group-boundary corrections).
  host: assemble per-row pos/den sums from row-accums, boundary reduces and
        colsums; subtract the exact diagonal exp(10*||z8||^2/256) computed
        from the shipped fp8 values; loss = mean(log den - log pos).
"""
import sys

sys.path.insert(0, "/opt/trn_rl_repo")
import numpy as np
import ml_dtypes

N, D, G, NCORES = 8192, 512, 4, 8
CT = 512               # supertile width
NT = N // CT           # 16 supertiles
SCALE = 16.0           # fp8 pre-scale
ESC = 10.0 / (SCALE * SCALE)   # exp scale applied to psum
F8NP = ml_dtypes.float8_e4m3

_built = None


def _build():
    from concourse import bacc, tile, mybir

    nc = bacc.Bacc(None, target_bir_lowering=False)
    f32 = mybir.dt.float32
    f8 = mybir.dt.float8e4
    AF = mybir.ActivationFunctionType
    AX = mybir.AxisListType
    PM = mybir.MatmulPerfMode.DoubleRow

    et = nc.dram_tensor("et", [128, 4, N], f8, kind="ExternalInput")
    ones2 = nc.dram_tensor("ones2", [128, 2, 16], f8, kind="ExternalInput")
    rsums = nc.dram_tensor("rsums", [128, 49], f32, kind="ExternalOutput")
    csum = nc.dram_tensor("csum", [15, 512], f32, kind="ExternalOutput")

    with tile.TileContext(nc) as tc:
        with tc.tile_pool(name="z", bufs=1) as zp, \
             tc.tile_pool(name="cst", bufs=1) as cst, \
             tc.tile_pool(name="eb", bufs=2) as ebp, \
             tc.tile_pool(name="pm", bufs=2, space="PSUM") as pmp, \
             tc.tile_pool(name="pc", bufs=2, space="PSUM") as pcp:

            nc.scalar.add_instruction(
                mybir.InstLoadActFuncSet(
                    name=nc.get_next_instruction_name(),
                    act_func_set_id=6, ins=[], outs=[]))

            o2 = cst.tile([128, 2, 16], f8)
            nc.sync.dma_start(o2[:], ones2[:])
            z8 = zp.tile([128, 4, N], f8)
            # progressive column chunks so compute starts after ~2.4us
            bounds = [0, CT, 2 * CT, 3 * CT, 6 * CT, 9 * CT, 12 * CT, 16 * CT]
            for lo, hi in zip(bounds, bounds[1:]):
                nc.sync.dma_start(z8[:, :, lo:hi], et[:, :, lo:hi])
            rsums_t = cst.tile([128, 49], f32)
            cstage = cst.tile([1, 15 * CT], f32)

            s = 0
            for Ii, I in enumerate((0, 8)):
                maxd = 9 if I == 0 else 8
                batches = [(0, 1, 2), (3, 4, 5),
                           (6, 7, 8) if I == 0 else (6, 7)]
                # exp tiles: [rb, d, col] fp8
                expt = ebp.tile([128, 4, 9, CT], f8, tag="expt")
                # batch-major: only the first pass over rb waits on new
                # input columns, later batches reuse columns already loaded
                for b, ds in enumerate(batches):
                    for rb in range(4):
                        rs = slice(I * CT + rb * 128, I * CT + (rb + 1) * 128)
                        pm = pmp.tile([128, 3 * CT], f32, tag="pm")
                        for di, d in enumerate(ds):
                            cs = slice((I + d) * CT, (I + d + 1) * CT)
                            for kk in range(2):
                                nc.tensor.matmul(
                                    pm[:, di * CT:(di + 1) * CT],
                                    z8[:, 2 * kk:2 * kk + 2, rs],
                                    z8[:, 2 * kk:2 * kk + 2, cs],
                                    start=(kk == 0), stop=(kk == 1),
                                    perf_mode=PM)
                        acol = Ii * 12 + rb * 3 + b
                        if I == 0 and rb == 0 and b == 0:
                            # split so the first exp only needs d0's columns
                            nc.scalar.activation(
                                expt[:, rb, 0:1, :], pm[:, :CT],
                                AF.Exp, scale=ESC,
                                accum_out=rsums_t[:, acol:acol + 1])
                            nc.scalar.activation(
                                expt[:, rb, 1:3, :], pm[:, CT:3 * CT],
                                AF.Exp, scale=ESC,
                                accum_out=rsums_t[:, 48:49])
                        else:
                            nc.scalar.activation(
                                expt[:, rb, ds[0]:ds[0] + len(ds), :],
                                pm[:, :len(ds) * CT], AF.Exp, scale=ESC,
                                accum_out=rsums_t[:, acol:acol + 1])
                    if b == 1:
                        # d=1..3 fp8 rowsums (group-boundary info)
                        for rb in range(4):
                            rcol = Ii * 12 + rb * 3
                            nc.vector.tensor_reduce(
                                rsums_t[:, 24 + rcol:24 + rcol + 3],
                                expt[:, rb, 1:4, :],
                                axis=AX.X, op=mybir.AluOpType.add)
                    # colsums for completed off-diag tiles of this batch
                    for d in ds:
                        if d == 0:
                            continue
                        pc = pcp.tile([1, CT], f32, tag="pc")
                        for h in range(2):
                            nc.tensor.matmul(
                                pc[:], o2[:, :, 0:1],
                                expt[:, 2 * h:2 * h + 2, d, :],
                                start=(h == 0), stop=(h == 1), perf_mode=PM)
                        if s == 13:
                            nc.scalar.copy(
                                cstage[:, s * CT:(s + 1) * CT], pc[:])
                        else:
                            nc.vector.tensor_copy(
                                cstage[:, s * CT:(s + 1) * CT], pc[:])
                        s += 1
                        if s == 13:
                            nc.sync.dma_start(csum[:13, :],
                                              cstage[:, :13 * CT])
            assert s == 15
            nc.sync.dma_start(csum[13:, :], cstage[:, 13 * CT:])
            nc.sync.dma_start(rsums[:], rsums_t[:])

    nc.finalize()
    return nc


def _get_built():
    global _built
    if _built is None:
        _built = _build()
    return _built


def _host_prep(embeddings, survival_times):
    E = np.ascontiguousarray(np.asarray(embeddings, dtype=np.float32))
    t = np.asarray(survival_times, dtype=np.float32)
    q = np.quantile(t.astype(np.float64), [0.25, 0.5, 0.75])
    rg = (t[:, None].astype(np.float64) >= q[None, :]).sum(axis=1)
    counts = np.bincount(rg, minlength=G)
    assert (counts == N // G).all(), counts
    perm = np.argsort(rg, kind="stable")
    Es = E[perm]
    nrm = np.sqrt((Es.astype(np.float64) ** 2).sum(axis=1, keepdims=True))
    z = Es / np.maximum(nrm, 1e-12)
    z16 = (z * SCALE).astype(F8NP)          # [N, D] fp8
    zT = np.ascontiguousarray(z16.T)        # [D, N]
    ones2 = np.zeros((128, 2, 16), dtype=F8NP)
    ones2[:, :, 0] = 1.0
    in_maps = []
    for c in range(NCORES):
        roll = np.roll(zT, -c * CT, axis=1)               # [D, N]
        et = np.ascontiguousarray(
            roll.reshape(4, 128, N).transpose(1, 0, 2))    # [128, 4, N]
        in_maps.append({"et": et, "ones2": ones2})
    return in_maps, z16


def _host_combine(results, z16):
    tot = np.zeros(N, np.float64)
    pos = np.zeros(N, np.float64)
    for c in range(NCORES):
        rs_ = results[c]["rsums"].astype(np.float64)
        racc, rred = rs_[:, :24].copy(), rs_[:, 24:48]
        racc[:, 0] += rs_[:, 48]     # (I=0, rb=0) batch0 was split in two
        csum = results[c]["csum"].astype(np.float64)
        s = 0
        for Ii, I in enumerate((0, 8)):
            aI = (I + c) % NT
            maxd = 9 if I == 0 else 8
            gI = aI // 4
            kp = 4 - (aI % 4)
            for rb in range(4):
                rows = slice(aI * CT + rb * 128, aI * CT + (rb + 1) * 128)
                A = racc[:, Ii * 12 + rb * 3: Ii * 12 + rb * 3 + 3]
                R = rred[:, Ii * 12 + rb * 3: Ii * 12 + rb * 3 + 3]
                tot[rows] += A.sum(axis=1)
                if kp == 1:
                    p = A[:, 0] - R[:, 0] - R[:, 1]
                elif kp == 2:
                    p = A[:, 0] - R[:, 1]
                elif kp == 3:
                    p = A[:, 0]
                else:
                    p = A[:, 0] + R[:, 2]
                pos[rows] += p
            for d in range(1, maxd):
                aJ = (I + d + c) % NT
                rows = slice(aJ * CT, (aJ + 1) * CT)
                tot[rows] += csum[s]
                if aJ // 4 == gI:
                    pos[rows] += csum[s]
                s += 1
    dlog = ESC * (z16.astype(np.float64) ** 2).sum(axis=1)
    dexp = np.exp(dlog)
    tot -= dexp
    pos -= dexp
    return np.float32(np.mean(np.log(tot) - np.log(pos)))


def kernel(embeddings, survival_times, censor):
    from concourse.bass_utils import run_bass_kernel_spmd

    nc = _get_built()
    in_maps, z16 = _host_prep(embeddings, survival_times)
    res = run_bass_kernel_spmd(nc, in_maps, list(range(NCORES)))
    return _host_combine(res.results, z16)


# revision 12
# speedup vs baseline: 3.7862x; 1.0283x over previous
"""Distributed Trainium2 kernel for nn_ContrastiveLoss (survival contrastive loss).

Strategy (8 NeuronCores, symmetric fp8):
  host: quantile-bin rows into 4 risk groups (2048 each), stable-sort by
        group, L2-normalize, scale by 16 and cast to fp8e4 (e4m3); ship a
        rolled copy to each core so its supertile-rows sit at fixed virtual
        positions (SPMD-static program).
  device (core c): sim is symmetric, so only supertile pairs (I, I+d) for
        virtual I in {0,8}, d = 0..8 / 0..7 are computed — over 8 rolled
        copies this covers all 136 unordered 512x512 supertile pairs once.
        fp8 DoubleRow matmuls (K=256/matmul) -> psum; ACT exp (scale 10/256)
        in 3-tile batches with f32 accum row-sums; fp8 exp tiles feed
        DoubleRow ones-matmul column-sums (the mirror contribution) and a
        DVE reduce of tiles d=1..3 (group-boundary corrections).
  host: assemble per-row pos/den sums from row-accums, boundary reduces and
        colsums; subtract the exact diagonal exp(10*||z8||^2/256) computed
        from the shipped fp8 values; loss = mean(log den - log pos).
"""
import sys

sys.path.insert(0, "/opt/trn_rl_repo")
import numpy as np
import ml_dtypes

N, D, G, NCORES = 8192, 512, 4, 8
CT = 512               # supertile width
NT = N // CT           # 16 supertiles
SCALE = 16.0           # fp8 pre-scale
ESC = 10.0 / (SCALE * SCALE)   # exp scale applied to psum
F8NP = ml_dtypes.float8_e4m3

_built = None


def _build():
    from concourse import bacc, tile, mybir

    nc = bacc.Bacc(None, target_bir_lowering=False)
    f32 = mybir.dt.float32
    f8 = mybir.dt.float8e4
    AF = mybir.ActivationFunctionType
    AX = mybir.AxisListType
    PM = mybir.MatmulPerfMode.DoubleRow

    et = nc.dram_tensor("et", [128, 4, N], f8, kind="ExternalInput")
    ones2 = nc.dram_tensor("ones2", [128, 2, 16], f8, kind="ExternalInput")
    rsums = nc.dram_tensor("rsums", [128, 49], f32, kind="ExternalOutput")
    csum = nc.dram_tensor("csum", [15, 512], f32, kind="ExternalOutput")

    with tile.TileContext(nc) as tc:
        with tc.tile_pool(name="z", bufs=1) as zp, \
             tc.tile_pool(name="cst", bufs=1) as cst, \
             tc.tile_pool(name="eb", bufs=2) as ebp, \
             tc.tile_pool(name="pm", bufs=2, space="PSUM") as pmp, \
             tc.tile_pool(name="pc", bufs=2, space="PSUM") as pcp:

            nc.scalar.add_instruction(
                mybir.InstLoadActFuncSet(
                    name=nc.get_next_instruction_name(),
                    act_func_set_id=6, ins=[], outs=[]))

            o2 = cst.tile([128, 2, 16], f8)
            z8 = zp.tile([128, 4, N], f8)
            # progressive column chunks so compute starts after ~3.6us;
            # ones2 (needed only for colsums ~10us in) goes mid-stream
            bounds = [0, CT, 2 * CT, 3 * CT, 6 * CT, 9 * CT, 12 * CT, 16 * CT]
            for i, (lo, hi) in enumerate(zip(bounds, bounds[1:])):
                nc.sync.dma_start(z8[:, :, lo:hi], et[:, :, lo:hi])
                if i == 2:
                    nc.sync.dma_start(o2[:], ones2[:])
            rsums_t = cst.tile([128, 49], f32)
            cstage = cst.tile([1, 15 * CT], f32)

            s = 0
            for Ii, I in enumerate((0, 8)):
                maxd = 9 if I == 0 else 8
                batches = [(0, 1, 2), (3, 4, 5),
                           (6, 7, 8) if I == 0 else (6, 7)]
                # exp tiles: [rb, d, col] fp8
                expt = ebp.tile([128, 4, 9, CT], f8, tag="expt")
                # batch-major: only the first pass over rb waits on new
                # input columns, later batches reuse columns already loaded
                for b, ds in enumerate(batches):
                    for rb in range(4):
                        rs = slice(I * CT + rb * 128, I * CT + (rb + 1) * 128)
                        pm = pmp.tile([128, 3 * CT], f32, tag="pm")
                        acol = Ii * 12 + rb * 3 + b
                        first = I == 0 and rb == 0 and b == 0
                        for di, d in enumerate(ds):
                            cs = slice((I + d) * CT, (I + d + 1) * CT)
                            for kk in range(2):
                                nc.tensor.matmul(
                                    pm[:, di * CT:(di + 1) * CT],
                                    z8[:, 2 * kk:2 * kk + 2, rs],
                                    z8[:, 2 * kk:2 * kk + 2, cs],
                                    start=(kk == 0), stop=(kk == 1),
                                    perf_mode=PM)
                            if first and di == 0:
                                # split: the first exp needs only d0's columns
                                nc.scalar.activation(
                                    expt[:, rb, 0:1, :], pm[:, :CT],
                                    AF.Exp, scale=ESC,
                                    accum_out=rsums_t[:, acol:acol + 1])
                        if first:
                            nc.scalar.activation(
                                expt[:, rb, 1:3, :], pm[:, CT:3 * CT],
                                AF.Exp, scale=ESC,
                                accum_out=rsums_t[:, 48:49])
                        else:
                            nc.scalar.activation(
                                expt[:, rb, ds[0]:ds[0] + len(ds), :],
                                pm[:, :len(ds) * CT], AF.Exp, scale=ESC,
                                accum_out=rsums_t[:, acol:acol + 1])
                    if b == 1:
                        # d=1..3 fp8 rowsums (group-boundary info)
                        for rb in range(4):
                            rcol = Ii * 12 + rb * 3
                            nc.vector.tensor_reduce(
                                rsums_t[:, 24 + rcol:24 + rcol + 3],
                                expt[:, rb, 1:4, :],
                                axis=AX.X, op=mybir.AluOpType.add)
                    # colsums for completed off-diag tiles of this batch
                    for d in ds:
                        if d == 0:
                            continue
                        pc = pcp.tile([1, CT], f32, tag="pc")
                        for h in range(2):
                            nc.tensor.matmul(
                                pc[:], o2[:, :, 0:1],
                                expt[:, 2 * h:2 * h + 2, d, :],
                                start=(h == 0), stop=(h == 1), perf_mode=PM)
                        if s == 13:
                            nc.scalar.copy(
                                cstage[:, s * CT:(s + 1) * CT], pc[:])
                        else:
                            nc.vector.tensor_copy(
                                cstage[:, s * CT:(s + 1) * CT], pc[:])
                        s += 1
                        if s == 13:
                            nc.sync.dma_start(csum[:13, :],
                                              cstage[:, :13 * CT])
            assert s == 15
            nc.sync.dma_start(rsums[:], rsums_t[:])
            nc.sync.dma_start(csum[13:, :], cstage[:, 13 * CT:])

    nc.finalize()
    return nc


def _get_built():
    global _built
    if _built is None:
        _built = _build()
    return _built


def _host_prep(embeddings, survival_times):
    E = np.ascontiguousarray(np.asarray(embeddings, dtype=np.float32))
    t = np.asarray(survival_times, dtype=np.float32)
    q = np.quantile(t.astype(np.float64), [0.25, 0.5, 0.75])
    rg = (t[:, None].astype(np.float64) >= q[None, :]).sum(axis=1)
    counts = np.bincount(rg, minlength=G)
    assert (counts == N // G).all(), counts
    perm = np.argsort(rg, kind="stable")
    Es = E[perm]
    nrm = np.sqrt((Es.astype(np.float64) ** 2).sum(axis=1, keepdims=True))
    z = Es / np.maximum(nrm, 1e-12)
    z16 = (z * SCALE).astype(F8NP)          # [N, D] fp8
    zT = np.ascontiguousarray(z16.T)        # [D, N]
    ones2 = np.zeros((128, 2, 16), dtype=F8NP)
    ones2[:, :, 0] = 1.0
    in_maps = []
    for c in range(NCORES):
        roll = np.roll(zT, -c * CT, axis=1)               # [D, N]
        et = np.ascontiguousarray(
            roll.reshape(4, 128, N).transpose(1, 0, 2))    # [128, 4, N]
        in_maps.append({"et": et, "ones2": ones2})
    return in_maps, z16


def _host_combine(results, z16):
    tot = np.zeros(N, np.float64)
    pos = np.zeros(N, np.float64)
    for c in range(NCORES):
        rs_ = results[c]["rsums"].astype(np.float64)
        racc, rred = rs_[:, :24].copy(), rs_[:, 24:48]
        racc[:, 0] += rs_[:, 48]     # (I=0, rb=0) batch0 was split in two
        csum = results[c]["csum"].astype(np.float64)
        s = 0
        for Ii, I in enumerate((0, 8)):
            aI = (I + c) % NT
            maxd = 9 if I == 0 else 8
            gI = aI // 4
            kp = 4 - (aI % 4)
            for rb in range(4):
                rows = slice(aI * CT + rb * 128, aI * CT + (rb + 1) * 128)
                A = racc[:, Ii * 12 + rb * 3: Ii * 12 + rb * 3 + 3]
                R = rred[:, Ii * 12 + rb * 3: Ii * 12 + rb * 3 + 3]
                tot[rows] += A.sum(axis=1)
                if kp == 1:
                    p = A[:, 0] - R[:, 0] - R[:, 1]
                elif kp == 2:
                    p = A[:, 0] - R[:, 1]
                elif kp == 3:
                    p = A[:, 0]
                else:
                    p = A[:, 0] + R[:, 2]
                pos[rows] += p
            for d in range(1, maxd):
                aJ = (I + d + c) % NT
                rows = slice(aJ * CT, (aJ + 1) * CT)
                tot[rows] += csum[s]
                if aJ // 4 == gI:
                    pos[rows] += csum[s]
                s += 1
    dlog = ESC * (z16.astype(np.float64) ** 2).sum(axis=1)
    dexp = np.exp(dlog)
    tot -= dexp
    pos -= dexp
    return np.float32(np.mean(np.log(tot) - np.log(pos)))


def kernel(embeddings, survival_times, censor):
    from concourse.bass_utils import run_bass_kernel_spmd

    nc = _get_built()
    in_maps, z16 = _host_prep(embeddings, survival_times)
    res = run_bass_kernel_spmd(nc, in_maps, list(range(NCORES)))
    return _host_combine(res.results, z16)


# revision 13
# speedup vs baseline: 3.8189x; 1.0086x over previous
"""Distributed Trainium2 kernel for nn_ContrastiveLoss (survival contrastive loss).

Strategy (8 NeuronCores, symmetric fp8):
  host: quantile-bin rows into 4 risk groups (2048 each), stable-sort by
        group, L2-normalize, scale by 16 and cast to fp8e4 (e4m3); ship a
        rolled copy to each core so its supertile-rows sit at fixed virtual
        positions (SPMD-static program).
  device (core c): sim is symmetric, so only supertile pairs (I, I+d) for
        virtual I in {0,8}, d = 0..8 / 0..7 are computed — over 8 rolled
        copies this covers all 136 unordered 512x512 supertile pairs once.
        fp8 DoubleRow matmuls (K=256/matmul) -> psum; ACT exp (scale 10/256)
        in 3-tile batches with f32 accum row-sums; fp8 exp tiles feed
        DoubleRow ones-matmul column-sums (the mirror contribution) and a
        DVE reduce of tiles d=1..3 (group-boundary corrections).
  host: assemble per-row pos/den sums from row-accums, boundary reduces and
        colsums; subtract the exact diagonal exp(10*||z8||^2/256) computed
        from the shipped fp8 values; loss = mean(log den - log pos).
"""
import sys

sys.path.insert(0, "/opt/trn_rl_repo")
import numpy as np
import ml_dtypes

N, D, G, NCORES = 8192, 512, 4, 8
CT = 512               # supertile width
NT = N // CT           # 16 supertiles
SCALE = 16.0           # fp8 pre-scale
ESC = 10.0 / (SCALE * SCALE)   # exp scale applied to psum
F8NP = ml_dtypes.float8_e4m3

_built = None


def _build():
    from concourse import bacc, tile, mybir

    nc = bacc.Bacc(None, target_bir_lowering=False)
    f32 = mybir.dt.float32
    f8 = mybir.dt.float8e4
    AF = mybir.ActivationFunctionType
    AX = mybir.AxisListType
    PM = mybir.MatmulPerfMode.DoubleRow

    et = nc.dram_tensor("et", [128, 4, N], f8, kind="ExternalInput")
    ones2 = nc.dram_tensor("ones2", [128, 2, 16], f8, kind="ExternalInput")
    rsums = nc.dram_tensor("rsums", [128, 49], f32, kind="ExternalOutput")
    csum = nc.dram_tensor("csum", [15, 512], f32, kind="ExternalOutput")

    with tile.TileContext(nc) as tc:
        with tc.tile_pool(name="z", bufs=1) as zp, \
             tc.tile_pool(name="cst", bufs=1) as cst, \
             tc.tile_pool(name="eb", bufs=2) as ebp, \
             tc.tile_pool(name="pm", bufs=2, space="PSUM") as pmp, \
             tc.tile_pool(name="pc", bufs=2, space="PSUM") as pcp:

            nc.scalar.add_instruction(
                mybir.InstLoadActFuncSet(
                    name=nc.get_next_instruction_name(),
                    act_func_set_id=6, ins=[], outs=[]))

            o2 = cst.tile([128, 2, 16], f8)
            z8 = zp.tile([128, 4, N], f8)
            # progressive column chunks so compute starts after ~3.6us;
            # ones2 (needed only for colsums ~10us in) goes mid-stream
            bounds = [0, CT, 3 * CT, 6 * CT, 9 * CT, 12 * CT, 16 * CT]
            for i, (lo, hi) in enumerate(zip(bounds, bounds[1:])):
                nc.sync.dma_start(z8[:, :, lo:hi], et[:, :, lo:hi])
                if i == 1:
                    nc.sync.dma_start(o2[:], ones2[:])
            rsums_t = cst.tile([128, 49], f32)
            cstage = cst.tile([1, 15 * CT], f32)

            s = 0
            for Ii, I in enumerate((0, 8)):
                maxd = 9 if I == 0 else 8
                batches = [(0, 1, 2), (3, 4, 5),
                           (6, 7, 8) if I == 0 else (6, 7)]
                # exp tiles: [rb, d, col] fp8
                expt = ebp.tile([128, 4, 9, CT], f8, tag="expt")
                # batch-major: only the first pass over rb waits on new
                # input columns, later batches reuse columns already loaded
                for b, ds in enumerate(batches):
                    for rb in range(4):
                        rs = slice(I * CT + rb * 128, I * CT + (rb + 1) * 128)
                        pm = pmp.tile([128, 3 * CT], f32, tag="pm")
                        acol = Ii * 12 + rb * 3 + b
                        first = I == 0 and rb == 0 and b == 0
                        for di, d in enumerate(ds):
                            cs = slice((I + d) * CT, (I + d + 1) * CT)
                            for kk in range(2):
                                nc.tensor.matmul(
                                    pm[:, di * CT:(di + 1) * CT],
                                    z8[:, 2 * kk:2 * kk + 2, rs],
                                    z8[:, 2 * kk:2 * kk + 2, cs],
                                    start=(kk == 0), stop=(kk == 1),
                                    perf_mode=PM)
                            if first and di == 0:
                                # split: the first exp needs only d0's columns
                                nc.scalar.activation(
                                    expt[:, rb, 0:1, :], pm[:, :CT],
                                    AF.Exp, scale=ESC,
                                    accum_out=rsums_t[:, acol:acol + 1])
                        if first:
                            nc.scalar.activation(
                                expt[:, rb, 1:3, :], pm[:, CT:3 * CT],
                                AF.Exp, scale=ESC,
                                accum_out=rsums_t[:, 48:49])
                        else:
                            nc.scalar.activation(
                                expt[:, rb, ds[0]:ds[0] + len(ds), :],
                                pm[:, :len(ds) * CT], AF.Exp, scale=ESC,
                                accum_out=rsums_t[:, acol:acol + 1])
                    if b == 1:
                        # d=1..3 fp8 rowsums (group-boundary info)
                        for rb in range(4):
                            rcol = Ii * 12 + rb * 3
                            nc.vector.tensor_reduce(
                                rsums_t[:, 24 + rcol:24 + rcol + 3],
                                expt[:, rb, 1:4, :],
                                axis=AX.X, op=mybir.AluOpType.add)
                    # colsums for completed off-diag tiles of this batch
                    for d in ds:
                        if d == 0:
                            continue
                        pc = pcp.tile([1, CT], f32, tag="pc")
                        for h in range(2):
                            nc.tensor.matmul(
                                pc[:], o2[:, :, 0:1],
                                expt[:, 2 * h:2 * h + 2, d, :],
                                start=(h == 0), stop=(h == 1), perf_mode=PM)
                        if s == 13:
                            nc.scalar.copy(
                                cstage[:, s * CT:(s + 1) * CT], pc[:])
                        else:
                            nc.vector.tensor_copy(
                                cstage[:, s * CT:(s + 1) * CT], pc[:])
                        s += 1
                        if s == 13:
                            nc.sync.dma_start(csum[:13, :],
                                              cstage[:, :13 * CT])
            assert s == 15
            nc.sync.dma_start(rsums[:], rsums_t[:])
            nc.sync.dma_start(csum[13:, :], cstage[:, 13 * CT:])

    nc.finalize()
    return nc


def _get_built():
    global _built
    if _built is None:
        _built = _build()
    return _built


def _host_prep(embeddings, survival_times):
    E = np.ascontiguousarray(np.asarray(embeddings, dtype=np.float32))
    t = np.asarray(survival_times, dtype=np.float32)
    q = np.quantile(t.astype(np.float64), [0.25, 0.5, 0.75])
    rg = (t[:, None].astype(np.float64) >= q[None, :]).sum(axis=1)
    counts = np.bincount(rg, minlength=G)
    assert (counts == N // G).all(), counts
    perm = np.argsort(rg, kind="stable")
    Es = E[perm]
    nrm = np.sqrt((Es.astype(np.float64) ** 2).sum(axis=1, keepdims=True))
    z = Es / np.maximum(nrm, 1e-12)
    z16 = (z * SCALE).astype(F8NP)          # [N, D] fp8
    zT = np.ascontiguousarray(z16.T)        # [D, N]
    ones2 = np.zeros((128, 2, 16), dtype=F8NP)
    ones2[:, :, 0] = 1.0
    in_maps = []
    for c in range(NCORES):
        roll = np.roll(zT, -c * CT, axis=1)               # [D, N]
        et = np.ascontiguousarray(
            roll.reshape(4, 128, N).transpose(1, 0, 2))    # [128, 4, N]
        in_maps.append({"et": et, "ones2": ones2})
    return in_maps, z16


def _host_combine(results, z16):
    tot = np.zeros(N, np.float64)
    pos = np.zeros(N, np.float64)
    for c in range(NCORES):
        rs_ = results[c]["rsums"].astype(np.float64)
        racc, rred = rs_[:, :24].copy(), rs_[:, 24:48]
        racc[:, 0] += rs_[:, 48]     # (I=0, rb=0) batch0 was split in two
        csum = results[c]["csum"].astype(np.float64)
        s = 0
        for Ii, I in enumerate((0, 8)):
            aI = (I + c) % NT
            maxd = 9 if I == 0 else 8
            gI = aI // 4
            kp = 4 - (aI % 4)
            for rb in range(4):
                rows = slice(aI * CT + rb * 128, aI * CT + (rb + 1) * 128)
                A = racc[:, Ii * 12 + rb * 3: Ii * 12 + rb * 3 + 3]
                R = rred[:, Ii * 12 + rb * 3: Ii * 12 + rb * 3 + 3]
                tot[rows] += A.sum(axis=1)
                if kp == 1:
                    p = A[:, 0] - R[:, 0] - R[:, 1]
                elif kp == 2:
                    p = A[:, 0] - R[:, 1]
                elif kp == 3:
                    p = A[:, 0]
                else:
                    p = A[:, 0] + R[:, 2]
                pos[rows] += p
            for d in range(1, maxd):
                aJ = (I + d + c) % NT
                rows = slice(aJ * CT, (aJ + 1) * CT)
                tot[rows] += csum[s]
                if aJ // 4 == gI:
                    pos[rows] += csum[s]
                s += 1
    dlog = ESC * (z16.astype(np.float64) ** 2).sum(axis=1)
    dexp = np.exp(dlog)
    tot -= dexp
    pos -= dexp
    return np.float32(np.mean(np.log(tot) - np.log(pos)))


def kernel(embeddings, survival_times, censor):
    from concourse.bass_utils import run_bass_kernel_spmd

    nc = _get_built()
    in_maps, z16 = _host_prep(embeddings, survival_times)
    res = run_bass_kernel_spmd(nc, in_maps, list(range(NCORES)))
    return _host_combine(res.results, z16)


# revision 14
# speedup vs baseline: 3.8531x; 1.0090x over previous
"""Distributed Trainium2 kernel for nn_ContrastiveLoss (survival contrastive loss).

Strategy (8 NeuronCores, symmetric fp8):
  host: quantile-bin rows into 4 risk groups (2048 each), stable-sort by
        group, L2-normalize, scale by 16 and cast to fp8e4 (e4m3); ship a
        rolled copy to each core so its supertile-rows sit at fixed virtual
        positions (SPMD-static program).
  device (core c): sim is symmetric, so only supertile pairs (I, I+d) for
        virtual I in {0,8}, d = 0..8 / 0..7 are computed — over 8 rolled
        copies this covers all 136 unordered 512x512 supertile pairs once.
        fp8 DoubleRow matmuls (K=256/matmul) -> psum; ACT exp (scale 10/256)
        in 3-tile batches with f32 accum row-sums; fp8 exp tiles feed
        DoubleRow ones-matmul column-sums (the mirror contribution) and a
        DVE reduce of tiles d=1..3 (group-boundary corrections).
  host: assemble per-row pos/den sums from row-accums, boundary reduces and
        colsums; subtract the exact diagonal exp(10*||z8||^2/256) computed
        from the shipped fp8 values; loss = mean(log den - log pos).
"""
import sys

sys.path.insert(0, "/opt/trn_rl_repo")
import numpy as np
import ml_dtypes

N, D, G, NCORES = 8192, 512, 4, 8
CT = 512               # supertile width
NT = N // CT           # 16 supertiles
SCALE = 16.0           # fp8 pre-scale
ESC = 10.0 / (SCALE * SCALE)   # exp scale applied to psum
F8NP = ml_dtypes.float8_e4m3

_built = None


def _build():
    from concourse import bacc, tile, mybir

    nc = bacc.Bacc(None, target_bir_lowering=False)
    f32 = mybir.dt.float32
    f8 = mybir.dt.float8e4
    AF = mybir.ActivationFunctionType
    AX = mybir.AxisListType
    PM = mybir.MatmulPerfMode.DoubleRow

    et = nc.dram_tensor("et", [128, 4, N], f8, kind="ExternalInput")
    ones2 = nc.dram_tensor("ones2", [128, 2, 16], f8, kind="ExternalInput")
    rsums = nc.dram_tensor("rsums", [128, 50], f32, kind="ExternalOutput")
    csum = nc.dram_tensor("csum", [15, 512], f32, kind="ExternalOutput")

    with tile.TileContext(nc) as tc:
        with tc.tile_pool(name="z", bufs=1) as zp, \
             tc.tile_pool(name="cst", bufs=1) as cst, \
             tc.tile_pool(name="eb", bufs=2) as ebp, \
             tc.tile_pool(name="pm", bufs=2, space="PSUM") as pmp, \
             tc.tile_pool(name="pc", bufs=2, space="PSUM") as pcp:

            nc.scalar.add_instruction(
                mybir.InstLoadActFuncSet(
                    name=nc.get_next_instruction_name(),
                    act_func_set_id=6, ins=[], outs=[]))

            o2 = cst.tile([128, 2, 16], f8)
            z8 = zp.tile([128, 4, N], f8)
            # progressive column chunks so compute starts after ~3.6us;
            # ones2 (needed only for colsums ~10us in) goes mid-stream
            bounds = [0, CT, 3 * CT, 6 * CT, 9 * CT, 12 * CT, 16 * CT]
            for i, (lo, hi) in enumerate(zip(bounds, bounds[1:])):
                nc.sync.dma_start(z8[:, :, lo:hi], et[:, :, lo:hi])
                if i == 1:
                    nc.sync.dma_start(o2[:], ones2[:])
            rsums_t = cst.tile([128, 50], f32)
            cstage = cst.tile([1, 15 * CT], f32)

            s = 0
            for Ii, I in enumerate((0, 8)):
                maxd = 9 if I == 0 else 8
                batches = [(0, 1, 2), (3, 4, 5),
                           (6, 7, 8) if I == 0 else (6, 7)]
                # exp tiles: [rb, d, col] fp8
                expt = ebp.tile([128, 4, 9, CT], f8, tag="expt")
                # batch-major: only the first pass over rb waits on new
                # input columns, later batches reuse columns already loaded
                for b, ds in enumerate(batches):
                    for rb in range(4):
                        rs = slice(I * CT + rb * 128, I * CT + (rb + 1) * 128)
                        pm = pmp.tile([128, 3 * CT], f32, tag="pm")
                        acol = Ii * 12 + rb * 3 + b
                        first = I == 0 and rb <= 1 and b == 0
                        for di, d in enumerate(ds):
                            cs = slice((I + d) * CT, (I + d + 1) * CT)
                            for kk in range(2):
                                nc.tensor.matmul(
                                    pm[:, di * CT:(di + 1) * CT],
                                    z8[:, 2 * kk:2 * kk + 2, rs],
                                    z8[:, 2 * kk:2 * kk + 2, cs],
                                    start=(kk == 0), stop=(kk == 1),
                                    perf_mode=PM)
                            if first and di == 0:
                                # split: the first exps need only d0's columns
                                nc.scalar.activation(
                                    expt[:, rb, 0:1, :], pm[:, :CT],
                                    AF.Exp, scale=ESC,
                                    accum_out=rsums_t[:, acol:acol + 1])
                        if first:
                            nc.scalar.activation(
                                expt[:, rb, 1:3, :], pm[:, CT:3 * CT],
                                AF.Exp, scale=ESC,
                                accum_out=rsums_t[:, 48 + rb:49 + rb])
                        else:
                            nc.scalar.activation(
                                expt[:, rb, ds[0]:ds[0] + len(ds), :],
                                pm[:, :len(ds) * CT], AF.Exp, scale=ESC,
                                accum_out=rsums_t[:, acol:acol + 1])
                    if b == 1:
                        # d=1..3 fp8 rowsums (group-boundary info)
                        for rb in range(4):
                            rcol = Ii * 12 + rb * 3
                            nc.vector.tensor_reduce(
                                rsums_t[:, 24 + rcol:24 + rcol + 3],
                                expt[:, rb, 1:4, :],
                                axis=AX.X, op=mybir.AluOpType.add)
                    # colsums for completed off-diag tiles of this batch
                    for d in ds:
                        if d == 0:
                            continue
                        pc = pcp.tile([1, CT], f32, tag="pc")
                        for h in range(2):
                            nc.tensor.matmul(
                                pc[:], o2[:, :, 0:1],
                                expt[:, 2 * h:2 * h + 2, d, :],
                                start=(h == 0), stop=(h == 1), perf_mode=PM)
                        if s == 13:
                            nc.scalar.copy(
                                cstage[:, s * CT:(s + 1) * CT], pc[:])
                        else:
                            nc.vector.tensor_copy(
                                cstage[:, s * CT:(s + 1) * CT], pc[:])
                        s += 1
                        if s == 13:
                            nc.sync.dma_start(csum[:13, :],
                                              cstage[:, :13 * CT])
            assert s == 15
            nc.sync.dma_start(rsums[:], rsums_t[:])
            nc.sync.dma_start(csum[13:, :], cstage[:, 13 * CT:])

    nc.finalize()
    return nc


def _get_built():
    global _built
    if _built is None:
        _built = _build()
    return _built


def _host_prep(embeddings, survival_times):
    E = np.ascontiguousarray(np.asarray(embeddings, dtype=np.float32))
    t = np.asarray(survival_times, dtype=np.float32)
    q = np.quantile(t.astype(np.float64), [0.25, 0.5, 0.75])
    rg = (t[:, None].astype(np.float64) >= q[None, :]).sum(axis=1)
    counts = np.bincount(rg, minlength=G)
    assert (counts == N // G).all(), counts
    perm = np.argsort(rg, kind="stable")
    Es = E[perm]
    nrm = np.sqrt((Es.astype(np.float64) ** 2).sum(axis=1, keepdims=True))
    z = Es / np.maximum(nrm, 1e-12)
    z16 = (z * SCALE).astype(F8NP)          # [N, D] fp8
    zT = np.ascontiguousarray(z16.T)        # [D, N]
    ones2 = np.zeros((128, 2, 16), dtype=F8NP)
    ones2[:, :, 0] = 1.0
    in_maps = []
    for c in range(NCORES):
        roll = np.roll(zT, -c * CT, axis=1)               # [D, N]
        et = np.ascontiguousarray(
            roll.reshape(4, 128, N).transpose(1, 0, 2))    # [128, 4, N]
        in_maps.append({"et": et, "ones2": ones2})
    return in_maps, z16


def _host_combine(results, z16):
    tot = np.zeros(N, np.float64)
    pos = np.zeros(N, np.float64)
    for c in range(NCORES):
        rs_ = results[c]["rsums"].astype(np.float64)
        racc, rred = rs_[:, :24].copy(), rs_[:, 24:48]
        racc[:, 0] += rs_[:, 48]     # (I=0, rb=0) batch0 was split in two
        racc[:, 3] += rs_[:, 49]     # (I=0, rb=1) batch0 was split in two
        csum = results[c]["csum"].astype(np.float64)
        s = 0
        for Ii, I in enumerate((0, 8)):
            aI = (I + c) % NT
            maxd = 9 if I == 0 else 8
            gI = aI // 4
            kp = 4 - (aI % 4)
            for rb in range(4):
                rows = slice(aI * CT + rb * 128, aI * CT + (rb + 1) * 128)
                A = racc[:, Ii * 12 + rb * 3: Ii * 12 + rb * 3 + 3]
                R = rred[:, Ii * 12 + rb * 3: Ii * 12 + rb * 3 + 3]
                tot[rows] += A.sum(axis=1)
                if kp == 1:
                    p = A[:, 0] - R[:, 0] - R[:, 1]
                elif kp == 2:
                    p = A[:, 0] - R[:, 1]
                elif kp == 3:
                    p = A[:, 0]
                else:
                    p = A[:, 0] + R[:, 2]
                pos[rows] += p
            for d in range(1, maxd):
                aJ = (I + d + c) % NT
                rows = slice(aJ * CT, (aJ + 1) * CT)
                tot[rows] += csum[s]
                if aJ // 4 == gI:
                    pos[rows] += csum[s]
                s += 1
    dlog = ESC * (z16.astype(np.float64) ** 2).sum(axis=1)
    dexp = np.exp(dlog)
    tot -= dexp
    pos -= dexp
    return np.float32(np.mean(np.log(tot) - np.log(pos)))


def kernel(embeddings, survival_times, censor):
    from concourse.bass_utils import run_bass_kernel_spmd

    nc = _get_built()
    in_maps, z16 = _host_prep(embeddings, survival_times)
    res = run_bass_kernel_spmd(nc, in_maps, list(range(NCORES)))
    return _host_combine(res.results, z16)
